# revision 38
# baseline (speedup 1.0000x reference)
"""Trainium2 Bass kernel for nn_MoEFusion (multi-modal MoE fusion MLP).

Data-parallel across 8 NeuronCores: batch dim (32768) sharded into 8
slices of 4096, all weights replicated. No collectives.

v3: fp8-e4m3 DoubleRow matmuls for the big GEMMs; biases folded into the
matmuls via a constant ones-row chunk; gate broadcast via a DRAM bounce
with a 128x-replicated read AP; pipeline keeps the PE stall-free so the
HAM clock gate stays at 2.4 GHz.

Per stripe (512 tokens), feature-major "T" layout:
  featT [3, 768, 512] fp8 --DMA--> ft[m] [128, 6, 512]
  px[m]  = proj_w8[m].T @ ft[m]          (3 fp8-DR MMs, K=768)
  xt[:,m]= fp8(px/64 + proj_b[m])        (ACT); xt[:,3] = const ones-row
  pg     = gate_w8.T @ xt (+gate_b row)  (2 DR MMs over xt slices 0-3)
  eT     = bf16 exp(pg/64)               (ACT Exp)
  sT     = colsum(eT)                    (PE ones-matmul colsum)
  rT     = 1/sT                          (DVE recip_approx_fast)
  gwT    = bf16(eT*rT)                   (DVE)
  gwT --DMA--> DRAM bounce --replicated DMA--> gb [128, 8, 512]
  ph[e]  = W1_8[e].T @ xt (+b1 row)      (2 DR MMs/expert)
  h[:,2i:2i+2] = bf16 relu(ph_pair/64)   (4 paired ACTs, bias-free)
  sh     = h * gb                        (ONE DVE TT over all 8 experts)
  pf     = b2x64.T @ gwT + sum_e W2x64[e].T @ sh[:,e]   (bf16 MMs)
  fT     = bf16(pf/64)  (DVE)  -> pen = relu(pre.T fT + pre_b) (PE+ACT)
  -> po = head.T pen (PE) -> out = po + head_b (DVE) --DMA--> outT

Software pipeline: the sh TT executes once gb lands (~one stripe after
the gate), and the l2/pre/head of stripe s-3 are emitted during stripe
s, so the PE never waits on the softmax/broadcast chain. PE warmup
matmuls run during the initial weight DMA so the HAM clock gate
un-throttles before real work arrives.
"""

import sys

if "/opt/trn_rl_repo" not in sys.path:
    sys.path.insert(0, "/opt/trn_rl_repo")

from contextlib import ExitStack

import ml_dtypes
import numpy as np

# ---- problem constants (hardcoded per contract) ----
B = 32768
NCORES = 8
BL = B // NCORES  # 4096 per core
STRIPE = 512
NSTRIPES = BL // STRIPE  # 8
NM = 3
NE = 8
D_IN = 768
KIN = D_IN // 128  # 6
D_P = 128
D_X = 384
KX = D_X // 128  # 3

WS = 64.0  # fp8 weight pre-scale
RWS = 1.0 / WS

BF16 = ml_dtypes.bfloat16
FP8 = ml_dtypes.float8_e4m3  # TRN FP8_EXP4 bit layout

# ---- f32 biases (columns of [128, WBCOLS]) ----
OFF_PROJB = 0
OFF_PREB = OFF_PROJB + NM
OFF_HEADB = OFF_PREB + 1
WBCOLS = OFF_HEADB + 1


def pack_weights(inp):
    """Host-side packing: fp8e4 x64 proj/W1/gate (with bias rows), bf16
    x64 W2/b2, bf16 pre/head, f32 proj/pre/head biases."""
    pw = np.asarray(inp["proj_w"], np.float32) * WS   # [3,768,128]
    wproj = pw.reshape(NM, KIN, 128, 128).transpose(2, 0, 1, 3)  # [128,3,6,128]

    w1 = np.asarray(inp["exp_w1"], np.float32) * WS   # [8,384,128]
    ww1 = np.zeros((128, NE, 4, 128), np.float32)
    ww1[:, :, :KX, :] = w1.reshape(NE, KX, 128, 128).transpose(2, 0, 1, 3)
    ww1[0, :, KX, :] = np.asarray(inp["exp_b1"], np.float32) * WS  # bias row

    gw = np.asarray(inp["gate_w"], np.float32) * WS   # [384,8]
    wg = np.zeros((128, 4, 16), np.float32)  # 16-col pad: DR step%16==0
    wg[:, :KX, :NE] = gw.reshape(KX, 128, NE).transpose(1, 0, 2)
    wg[0, KX, :NE] = np.asarray(inp["gate_b"], np.float32) * WS

    w2 = np.asarray(inp["exp_w2"], np.float32) * WS   # [8,128,128]
    ww2 = w2.transpose(1, 0, 2)                       # [128,8,128]
    b2 = np.asarray(inp["exp_b2"], np.float32) * WS   # [8,128]
    wpre = np.asarray(inp["pre_w"], np.float32)       # [128,64]
    whead = np.asarray(inp["head_w"], np.float32)     # [64,2]

    ones8 = np.zeros((128, STRIPE), np.float32)
    ones8[0, :] = 1.0
    onesg = np.ones((NE, NE), np.float32)

    wbias = np.zeros((128, WBCOLS), np.float32)
    wbias[:, OFF_PROJB:OFF_PROJB + NM] = np.asarray(inp["proj_b"], np.float32).T
    wbias[:64, OFF_PREB] = np.asarray(inp["pre_b"], np.float32)
    wbias[:2, OFF_HEADB] = np.asarray(inp["head_b"], np.float32)

    return {
        "wproj": np.ascontiguousarray(wproj).astype(FP8),
        "ww1": np.ascontiguousarray(ww1).astype(FP8),
        "wg": np.ascontiguousarray(wg).astype(FP8),
        "ones8": ones8.astype(FP8),
        "onesg": onesg.astype(BF16),
        "ww2": np.ascontiguousarray(ww2).astype(BF16),
        "wb2": b2.astype(BF16),
        "wpre": wpre.astype(BF16),
        "whead": whead.astype(BF16),
        "wbias": wbias,
    }


def build_program(n_stripes=NSTRIPES):
    import concourse.bacc as bacc
    import concourse.bass_isa as bass_isa
    import concourse.mybir as mybir
    import concourse.tile as tile

    f32 = mybir.dt.float32
    bf16 = mybir.dt.bfloat16
    fp8 = mybir.dt.float8e4
    AF = mybir.ActivationFunctionType
    ALU = mybir.AluOpType
    DR = mybir.MatmulPerfMode.DoubleRow
    bl = n_stripes * STRIPE

    nc = bacc.Bacc(
        "TRN2", target_bir_lowering=False, debug=False, enable_asserts=False
    )

    featT = nc.dram_tensor("featT", [NM, D_IN, bl], fp8, kind="ExternalInput").ap()
    wproj_d = nc.dram_tensor("wproj", [128, NM, KIN, 128], fp8, kind="ExternalInput").ap()
    ww1_d = nc.dram_tensor("ww1", [128, NE, 4, 128], fp8, kind="ExternalInput").ap()
    wg_d = nc.dram_tensor("wg", [128, 4, 16], fp8, kind="ExternalInput").ap()
    ones8_d = nc.dram_tensor("ones8", [128, STRIPE], fp8, kind="ExternalInput").ap()
    onesg_d = nc.dram_tensor("onesg", [NE, NE], bf16, kind="ExternalInput").ap()
    ww2_d = nc.dram_tensor("ww2", [128, NE, 128], bf16, kind="ExternalInput").ap()
    wb2_d = nc.dram_tensor("wb2", [NE, 128], bf16, kind="ExternalInput").ap()
    wpre_d = nc.dram_tensor("wpre", [128, 64], bf16, kind="ExternalInput").ap()
    whead_d = nc.dram_tensor("whead", [64, 2], bf16, kind="ExternalInput").ap()
    wbias_d = nc.dram_tensor("wbias", [128, WBCOLS], f32, kind="ExternalInput").ap()
    gwb_d = nc.dram_tensor("gwb", [n_stripes, NE, STRIPE], bf16, kind="Internal").ap()
    outT = nc.dram_tensor("outT", [2, bl], f32, kind="ExternalOutput").ap()

    with tile.TileContext(nc) as tc, ExitStack() as ctx:
        wp_pool = ctx.enter_context(tc.tile_pool(name="wp", bufs=1))
        feat_pool = ctx.enter_context(tc.tile_pool(name="feat", bufs=15))
        x_pool = ctx.enter_context(tc.tile_pool(name="x", bufs=3))
        gw_pool = ctx.enter_context(tc.tile_pool(name="gw", bufs=4))
        gb_pool = ctx.enter_context(tc.tile_pool(name="gb", bufs=3))
        h_pool = ctx.enter_context(tc.tile_pool(name="h", bufs=2))
        sh_pool = ctx.enter_context(tc.tile_pool(name="sh", bufs=4))
        f_pool = ctx.enter_context(tc.tile_pool(name="f", bufs=2))
        pen_pool = ctx.enter_context(tc.tile_pool(name="pen", bufs=4))
        o_pool = ctx.enter_context(tc.tile_pool(name="o", bufs=4))

        px_pool = ctx.enter_context(tc.tile_pool(name="px", bufs=2, space="PSUM"))
        ph_pool = ctx.enter_context(tc.tile_pool(name="ph", bufs=2, space="PSUM"))
        pf_pool = ctx.enter_context(tc.tile_pool(name="pf", bufs=1, space="PSUM"))
        ps_pool = ctx.enter_context(tc.tile_pool(name="ps", bufs=1, space="PSUM"))

        # ---- PE warmup: junk matmuls during weight DMA keep/raise the
        # HAM activity so real matmuls start at 2.4 GHz ----
        warm = wp_pool.tile([1, 64], bf16)
        nc.vector.memset(warm[:], 0.0)
        pwarm = ps_pool.tile([64, 64], f32, tag="ps")
        for _ in range(36):
            nc.tensor.matmul(pwarm[:], warm[:], warm[:], start=True, stop=True)

        # ---- preload weights; critical-path first per ring ----
        # scalar ring: fp8 weights (proj modality 0 first), then bf16.
        # sync ring: biases, const row, then per-stripe features.
        Bz = wp_pool.tile([128, WBCOLS], f32)
        nc.sync.dma_start(Bz[:], wbias_d[:])
        Og = wp_pool.tile([NE, NE], bf16)
        nc.sync.dma_start(Og[:], onesg_d[:])
        Wproj = wp_pool.tile([128, NM, KIN, 128], fp8)
        nc.scalar.dma_start(Wproj[:, 0:1], wproj_d[:, 0:1])
        nc.scalar.dma_start(Wproj[:, 1:3], wproj_d[:, 1:3])
        Wg = wp_pool.tile([128, 4, 16], fp8)
        nc.scalar.dma_start(Wg[:], wg_d[:])
        W1t = wp_pool.tile([128, NE, 4, 128], fp8)
        nc.scalar.dma_start(W1t[:, 0:4], ww1_d[:, 0:4])
        nc.scalar.dma_start(W1t[:, 4:8], ww1_d[:, 4:8])
        W2t = wp_pool.tile([128, NE, 128], bf16)
        nc.scalar.dma_start(W2t[:], ww2_d[:])
        B2t = wp_pool.tile([NE, 128], bf16)
        nc.scalar.dma_start(B2t[:], wb2_d[:])
        Wpre = wp_pool.tile([128, 64], bf16)
        nc.scalar.dma_start(Wpre[:], wpre_d[:])
        Whead = wp_pool.tile([64, 2], bf16)
        nc.scalar.dma_start(Whead[:], whead_d[:])

        def bslice(off, parts=128):
            return Bz[:parts, off : off + 1]

        featT_t = featT.rearrange("m (k p) b -> m p k b", p=128)

        def rep128(ap):
            """DRAM AP -> same AP read 128x (replicated partition fill)."""
            return type(ap)(
                ap.tensor, ap.offset, [[0, 128]] + [list(d) for d in ap.ap]
            )

        pends = []      # (sh, gwT, bsl) awaiting l2 (depth 3)
        sh_pend = None  # (h, gb_slot, sh_tile...) awaiting the gating TT
        head_q = []  # (pen, bsl) awaiting head matmuls (2-stripe delay)

        def emit_l2(pend):
            sh, gwT, bsl = pend
            pf = pf_pool.tile([128, STRIPE], f32, tag="pf")
            nc.tensor.matmul(pf[:], B2t[:, :], gwT[:], start=True, stop=False)
            for e in range(NE):
                nc.tensor.matmul(
                    pf[:], W2t[:, e, :], sh[:, e, :],
                    start=False, stop=(e == NE - 1),
                )
            fT = f_pool.tile([128, STRIPE], bf16, tag="f")
            nc.vector.tensor_scalar_mul(fT[:], pf[:], RWS)
            return fT

        def emit_pre(fT):
            pp = ps_pool.tile([64, STRIPE], f32, tag="ps")
            nc.tensor.matmul(pp[:], Wpre[:, :], fT[:], start=True, stop=True)
            pen = pen_pool.tile([64, STRIPE], bf16, tag="pen")
            nc.scalar.activation(
                pen[:], pp[:], AF.Relu, bias=bslice(OFF_PREB, parts=64), scale=1.0
            )
            return pen

        def emit_head2(pen, bsl):
            po = ps_pool.tile([2, STRIPE], f32, tag="ps")
            nc.tensor.matmul(po[:], Whead[:, :], pen[:], start=True, stop=True)
            ot = o_pool.tile([2, STRIPE], f32, tag="o")
            nc.vector.tensor_scalar(
                ot[:], po[:], bslice(OFF_HEADB, parts=2), None, op0=ALU.add
            )
            nc.sync.dma_start(outT[:, bsl], ot[:])

        def emit_sh(pend):
            h, gb = pend
            sh = sh_pool.tile([128, NE, STRIPE], bf16, tag="sh")
            nc.vector.tensor_mul(sh[:], h[:], gb[:])
            return sh

        def load_feats(s):
            fsl = slice(s * STRIPE, (s + 1) * STRIPE)
            ft = []
            for m in range(NM):
                t = feat_pool.tile([128, KIN, STRIPE], fp8, tag="feat")
                nc.sync.dma_start(t[:], featT_t[m, :, :, fsl])
                ft.append(t)
            return ft

        # features prefetched 3 stripes ahead so early stripes never
        # outrun the sync DMA queue
        feat_q = [load_feats(0), load_feats(1), load_feats(2)]

        for s in range(n_stripes):
            bsl = slice(s * STRIPE, (s + 1) * STRIPE)

            # ---- features (prefetched) + const row ----
            if s + 3 < n_stripes:
                feat_q.append(load_feats(s + 3))
            ft = feat_q.pop(0)
            xt = x_pool.tile([128, 4, STRIPE], fp8, tag="x")
            nc.vector.memset(xt[:, KX, :], 0.0)
            nc.vector.memset(xt[0:1, KX, :], 1.0)

            # ---- projections -> xt slices ----
            for m in range(NM):
                px = px_pool.tile([128, STRIPE], f32, tag="px")
                for j in range(KIN // 2):
                    nc.tensor.matmul(
                        px[:],
                        Wproj[:, m, 2 * j : 2 * j + 2, :],
                        ft[m][:, 2 * j : 2 * j + 2, :],
                        start=(j == 0),
                        stop=(j == KIN // 2 - 1),
                        perf_mode=DR,
                    )
                nc.scalar.activation(
                    xt[:, m, :], px[:], AF.Identity,
                    bias=bslice(OFF_PROJB + m), scale=RWS,
                )

            # ---- finish head of an older stripe (extra slack for pen) ----
            if len(head_q) == 2:
                emit_head2(*head_q.pop(0))

            # ---- l2 of stripe s-3 ----
            fT_prev = None
            if len(pends) == 3:
                p0 = pends.pop(0)
                fT_prev = emit_l2(p0)
                pend_bsl = p0[2]

            # ---- gate softmax (gate_b folded into DR pair 1) ----
            pg = ps_pool.tile([8, STRIPE], f32, tag="ps")
            nc.tensor.matmul(pg[:], Wg[:, 0:2, :NE], xt[:, 0:2, :],
                             start=True, stop=False, perf_mode=DR)
            nc.tensor.matmul(pg[:], Wg[:, 2:4, :NE], xt[:, 2:4, :],
                             start=False, stop=True, perf_mode=DR)
            eT = gw_pool.tile([8, STRIPE], bf16, tag="eT")
            nc.scalar.activation(eT[:], pg[:], AF.Exp, bias=0.0, scale=RWS)
            psum_s = ps_pool.tile([8, STRIPE], f32, tag="ps")
            nc.tensor.matmul(psum_s[:], Og[:, :], eT[:], start=True, stop=True)
            rT = gw_pool.tile([8, STRIPE], f32, tag="rT")
            nc.vector.reciprocal_approx_fast(rT[:], psum_s[:])
            gwT = gw_pool.tile([8, STRIPE], bf16, tag="gwT")
            nc.vector.tensor_mul(gwT[:], eT[:], rT[:])

            # gate rows -> DRAM bounce -> replicated read into gb
            nc.gpsimd.dma_start(gwb_d[s], gwT[:])
            gb = gb_pool.tile([128, NE, STRIPE], bf16, tag="gb")
            nc.gpsimd.dma_start(gb[:], rep128(gwb_d[s]))

            # ---- experts: ph pairs + bias-free relu ACTs ----
            h = h_pool.tile([128, NE, STRIPE], bf16, tag="h")
            for i in range(NE // 2):
                php = ph_pool.tile([128, 2, STRIPE], f32, tag="ph")
                for half in range(2):
                    e = 2 * i + half
                    nc.tensor.matmul(
                        php[:, half, :], W1t[:, e, 0:2, :], xt[:, 0:2, :],
                        start=True, stop=False, perf_mode=DR,
                    )
                    nc.tensor.matmul(
                        php[:, half, :], W1t[:, e, 2:4, :], xt[:, 2:4, :],
                        start=False, stop=True, perf_mode=DR,
                    )
                nc.scalar.activation(
                    h[:, 2 * i : 2 * i + 2, :], php[:], AF.Relu,
                    bias=0.0, scale=RWS,
                )

            pends.append((emit_sh((h, gb)), gwT, bsl))

            if fT_prev is not None:
                head_q.append((emit_pre(fT_prev), pend_bsl))

        # ---- drain ----
        for hp in head_q:
            emit_head2(*hp)
        for p0 in pends:
            fT = emit_l2(p0)
            pen = emit_pre(fT)
            emit_head2(pen, p0[2])

    nc.compile()
    return nc


_PROGRAM = None


def _get_program():
    global _PROGRAM
    if _PROGRAM is None:
        _PROGRAM = build_program()
    return _PROGRAM


def make_in_maps(inputs):
    """Host-side shard + layout prep: list of 8 per-core input maps."""
    w = pack_weights(inputs)
    feats = [
        np.asarray(inputs["feat_text"], np.float32),
        np.asarray(inputs["feat_audio"], np.float32),
        np.asarray(inputs["feat_video"], np.float32),
    ]
    in_maps = []
    for c in range(NCORES):
        sl = slice(c * BL, (c + 1) * BL)
        featT = np.stack([np.ascontiguousarray(f[sl].T) for f in feats])
        in_maps.append({"featT": featT.astype(FP8), **w})
    return in_maps


def run_on_hw(inputs, trace=False):
    from concourse.bass_utils import run_bass_kernel_spmd

    nc = _get_program()
    in_maps = make_in_maps(inputs)
    res = run_bass_kernel_spmd(
        nc, in_maps, core_ids=list(range(NCORES)), trace=trace
    )
    out = np.concatenate([r["outT"].T for r in res.results], axis=0)
    return out, res


def kernel(**inputs):
    out, _ = run_on_hw(inputs, trace=False)
    return out
